# revision 56
# baseline (speedup 1.0000x reference)
"""Trainium2 Bass kernel for GQA attention (B=2, S=2048, DIM=2048, H=16, KV=8,
HD=128) with RoPE + causal mask + output projection.

Sharding: 8-way tensor parallelism over heads. Core c computes q heads
{2c, 2c+1} and kv head c end-to-end (QKV projection, RoPE, causal attention),
contributes its transposed attention output to on-device AllGathers, then
computes the output-projection column slice out[:, 256c:256(c+1)] from the
gathered activations. The host only slices inputs and concatenates outputs.

Schedule (v6; driven by trace analysis - NOTE this fleet runs the PE under
a ~50% duty-cap governor, so total PE work, not packing, sets wall-clock):

  proj(b0) -> attn(b0: t0..t3, AG fired per chunk, finalize deferred one
  chunk) -> proj(b1) [wo(b0 chunks) interleaved between windows]
  -> attn(b1: t3..t0) -> tail wo(b1 chunks)

The first AllGather fires right after proj(b0)+one small chunk (~110us) and
the 8 AGs overlap proj(b1)/attn(b1). b0 runs small-first so its AG pipeline
starts earliest; b1 runs big-first so the tail waits only small-chunk AGs.

Key points:
- softmax denominator = DVE bf16 2x-mode accumulation of exp tiles (eacc),
  then one ones^T @ eacc PE matmul per head into a dedicated PSUM bank row
  (a shared-bank row-offset variant is numerically broken - micro_psum.py).
  The exact [1,512] DVE reciprocal costs ~3.3us (cost scales with elements,
  so a broadcast-then-[128,512]-reciprocal variant is far worse);
  reciprocal_approx_fast (~18 correct bits, ~5x faster) is used instead,
  and the finalize (PE row-broadcast, ACT copy, DVE normalize, AG fire) is
  deferred into the next chunk's idx==1 slot to cover the chain.
- causal-masked columns of diagonal blocks are skipped end-to-end: scores /
  exp / eacc-add / AV all operate on the [128*rel:512] column subset
  (per-element PSUM has_written semantics keep the accumulations correct;
  micro_psum.py test B validates the subset-start pattern).
- causal mask as subset-width -30 bias matmuls before the scores matmul
  (start=True clears the whole bank).
- RoPE as 6 bf16 2x-mode DVE ops per head on ACT-evicted bf16 copies of
  the projection PSUM, with [cos;sin]/[sin;cos] stacked trig tables.
- quarter-batched DMAs (one trigger per [128,4,512]): xt stream on the sync
  queue, gathered wo chunks on the scalar queue. Engine queues are kept
  decoupled: POOL carries only AG doorbells, ag_in stores ride sync, out
  stores ride scalar - anything after a doorbell on POOL waits on the
  collective and must not gate compute.
- AG outputs addr_space="Shared"; output stored bf16, host upcasts.
- single set of qrot/krot/vnat buffers reused by both batches (b0's
  attention completes before proj(b1) overwrites them).
- eacc accumulators for chunk i+1 are allocated+zeroed mid-chunk-i on DVE
  so the memsets sit off the chunk-boundary dependency chain.
"""

import sys

if "/opt/trn_rl_repo" not in sys.path:
    sys.path.insert(0, "/opt/trn_rl_repo")

import numpy as np
import ml_dtypes

B, S, DIM = 2, 2048, 2048
H, KV, HD = 16, 8, 128
NC = 8
NS = B * S            # 4096 flattened (b, s) rows
P = 128
MB = DIM // P         # 16 contraction blocks for the projections
BF = ml_dtypes.bfloat16

_cache: dict = {}


def _build(debug=False):
    import concourse.bass as bass
    import concourse.mybir as mybir
    import concourse.tile as tile
    from concourse import bacc
    from concourse.masks import make_identity

    dt = mybir.dt
    f32, bf16 = dt.float32, dt.bfloat16
    Exp = mybir.ActivationFunctionType.Exp

    nc = bacc.Bacc("TRN2", debug=False, target_bir_lowering=False, num_devices=NC)

    xT_h = nc.dram_tensor("xT", (MB, 8, P, 512), bf16, kind="ExternalInput").ap()
    wq_h = nc.dram_tensor("wq_c", (P, MB * 256), bf16, kind="ExternalInput").ap()
    wk_h = nc.dram_tensor("wk_c", (P, MB * HD), bf16, kind="ExternalInput").ap()
    wv_h = nc.dram_tensor("wv_c", (P, MB * HD), bf16, kind="ExternalInput").ap()
    wo_h = nc.dram_tensor("wo_c", (P, MB * 256), bf16, kind="ExternalInput").ap()
    # stacked trig tables: T1 = [cos; sin], T2 = [sin; cos]  (128 x NS)
    t1_h = nc.dram_tensor("trig1", (P, NS), bf16, kind="ExternalInput").ap()
    t2_h = nc.dram_tensor("trig2", (P, NS), bf16, kind="ExternalInput").ap()
    mskb_h = nc.dram_tensor("maskb", (P, 4 * 512), bf16, kind="ExternalInput").ap()
    out_h = nc.dram_tensor("outT", (256, NS), bf16, kind="ExternalOutput").ap()

    with tile.TileContext(nc) as tc:
        with (
            tc.tile_pool(name="const", bufs=1) as const,
            tc.tile_pool(name="persist", bufs=1) as persist,
            tc.tile_pool(name="xs", bufs=2) as xs,
            tc.tile_pool(name="vt", bufs=2) as vt,
            tc.tile_pool(name="mt", bufs=2) as mt,
            tc.tile_pool(name="et", bufs=20) as et,
            tc.tile_pool(name="rt", bufs=2) as rt,
            tc.tile_pool(name="ov", bufs=2) as ov,
            tc.tile_pool(name="gp", bufs=2) as gp,
            tc.tile_pool(name="ot", bufs=3) as ot,
            tc.tile_pool(name="dram", bufs=1, space="DRAM") as dram,
            tc.tile_pool(name="psW", bufs=1, space="PSUM") as psW,
        ):
            # ---- constants into SBUF ----
            # wq in halves: the second half loads behind the first xt
            # quarters so window 0's matmuls start ~8us earlier
            wq_sb = [const.tile([P, 8, 256], bf16, name=f"wq{i}")
                     for i in range(2)]
            nc.sync.dma_start(
                wq_sb[0][:],
                wq_h[:, 0:8 * 256].rearrange("p (mb d) -> p mb d", mb=8))
            wk_sb = const.tile([P, MB, HD], bf16)
            nc.scalar.dma_start(wk_sb[:], wk_h.rearrange("p (mb d) -> p mb d", mb=MB))
            wv_sb = const.tile([P, MB, HD], bf16)
            nc.gpsimd.dma_start(wv_sb[:], wv_h.rearrange("p (mb d) -> p mb d", mb=MB))
            t1_sb = const.tile([P, NS], bf16)
            nc.scalar.dma_start(t1_sb[:], t1_h)
            t2_sb = const.tile([P, NS], bf16)
            nc.scalar.dma_start(t2_sb[:], t2_h)
            wo_sb = const.tile([P, MB, 256], bf16)
            mskb_sb = const.tile([P, 4 * 512], bf16)
            nc.gpsimd.dma_start(mskb_sb[:], mskb_h)
            ones_sb = const.tile([P, 1], bf16)
            nc.gpsimd.memset(ones_sb[:], 1.0)
            ones_row = const.tile([1, P], bf16)
            nc.gpsimd.memset(ones_row[:], 1.0)
            ident = const.tile([P, P], bf16)
            make_identity(nc, ident[:])

            # ---- single activation set, reused by both batches ----
            qrot = persist.tile([P, 2, S], bf16, name="qrot")
            krot = persist.tile([P, S], bf16, name="krot")
            vnat = persist.tile([P, S // P, HD], bf16, name="vnat")
            ag_in = [[dram.tile([256, 512], bf16, name=f"agi{b}{t}")
                      for t in range(4)] for b in range(B)]
            ag_out = [[dram.tile([NC * 256, 512], bf16, name=f"ago{b}{t}",
                                 addr_space="Shared")
                       for t in range(4)] for b in range(B)]

            fired = []       # chunks whose AllGather has been emitted
            wo_pending = []  # chunks gathered but whose wo is not yet emitted

            def emit_wo_one():
                """Output-projection slice for the oldest gathered chunk."""
                b_w, t_w = wo_pending.pop(0)
                # quarter loads so the first wo matmuls start after 512KB
                gq = []
                src = ag_out[b_w][t_w].rearrange("(r p) q -> p r q", p=P)
                for q4 in range(4):
                    g = gp.tile([P, 4, 512], bf16, tag=f"g{q4}", name=f"g{q4}")
                    nc.scalar.dma_start(g[:], src[:, q4 * 4:(q4 + 1) * 4])
                    gq.append(g)
                # two passes over one PSUM bank (PSUM budget is 8 banks)
                for n in range(2):
                    pw = psW.tile([P, 512], f32, tag="pw", name="pw")
                    for r in range(MB):
                        nc.tensor.matmul(
                            pw[:], wo_sb[:, r, n * 128:(n + 1) * 128],
                            gq[r // 4][:, r % 4, :],
                            start=(r == 0), stop=(r == MB - 1),
                        )
                    o = ot.tile([P, 512], bf16, tag="o", name="o")
                    nc.vector.tensor_copy(o[:], pw[:])
                    nc.scalar.dma_start(
                        out_h[n * P:(n + 1) * P,
                              b_w * S + t_w * 512: b_w * S + (t_w + 1) * 512],
                        o[:],
                    )

            def drain_wo(min_pending):
                """Emit at most one wo block, only if more than min_pending
                chunks are queued (i.e. the head chunk's AllGather has had
                min_pending chunks of compute time to complete)."""
                if len(wo_pending) > min_pending:
                    emit_wo_one()

            def proj_batch(b, psA2, psA1, psT, wo_lag=None):
                for sp in range(4):          # 512-col windows within batch
                    gw = slice(b * S + sp * 512, b * S + (sp + 1) * 512)
                    lw = slice(sp * 512, (sp + 1) * 512)
                    # quarter loads so matmuls stream behind the DMA
                    xsrc = xT_h[:, b * 4 + sp].rearrange("m p q -> p m q")
                    xtq = []
                    for q4 in range(4):
                        xt = xs.tile([P, 4, 512], bf16, tag=f"xt{q4}",
                                     name=f"xt{q4}")
                        nc.sync.dma_start(xt[:], xsrc[:, q4 * 4:(q4 + 1) * 4])
                        xtq.append(xt)
                        if b == 0 and sp == 0 and q4 == 1:
                            nc.sync.dma_start(
                                wq_sb[1][:],
                                wq_h[:, 8 * 256:].rearrange(
                                    "p (mb d) -> p mb d", mb=8))
                    pq = [psA2.tile([P, 512], f32, tag=f"pq{h}", name=f"pq{h}")
                          for h in range(2)]
                    pk = psA1.tile([P, 512], f32, tag="pk", name="pk")
                    pv = psA1.tile([P, 512], f32, tag="pv", name="pv")
                    for m in range(MB):
                        wq_m = wq_sb[m // 8]
                        for acc, lhsT in (
                            (pq[0], wq_m[:, m % 8, 0:128]),
                            (pq[1], wq_m[:, m % 8, 128:256]),
                            (pk, wk_sb[:, m, :]),
                            (pv, wv_sb[:, m, :]),
                        ):
                            nc.tensor.matmul(
                                acc[:], lhsT, xtq[m // 4][:, m % 4, :],
                                start=(m == 0), stop=(m == MB - 1),
                            )
                    # PSUM -> SBUF bf16 evictions on ACT
                    pq_bf = [mt.tile([P, 512], bf16, tag=f"pqb{h}",
                                     name=f"pqb{h}") for h in range(2)]
                    pk_bf = mt.tile([P, 512], bf16, tag="pkb", name="pkb")
                    for h in range(2):
                        nc.scalar.copy(pq_bf[h][:], pq[h][:])
                    nc.scalar.copy(pk_bf[:], pk[:])
                    vT_w = vt.tile([P, 512], bf16, tag="vtw", name="vtw")
                    nc.scalar.copy(vT_w[:], pv[:])
                    # RoPE on DVE (all-bf16 2x ops)
                    for h in range(2):
                        rope_unit(pq_bf[h], gw,
                                  qrot[0:64, h, lw], qrot[64:128, h, lw])
                    rope_unit(pk_bf, gw, krot[0:64, lw], krot[64:128, lw])
                    # v natural layout: PE transposes of this window's blocks
                    for i4 in range(4):
                        blk = sp * 4 + i4
                        pt = psT.tile([P, P], bf16, tag="pt", name="pt")
                        nc.tensor.transpose(
                            pt[:], vT_w[:, i4 * P:(i4 + 1) * P], ident[:])
                        nc.vector.tensor_copy(vnat[:, blk, :], pt[:])
                    if wo_lag is not None:
                        drain_wo(0)

            def rope_unit(src_bf, cols, out_even, out_odd):
                # src_bf rows 0:64 = even dims E, 64:128 = odd dims O.
                # rotE = E*cos - O*sin, rotO = E*sin + O*cos.
                # t1 = [cos;sin], t2 = [sin;cos]; products go to base-0
                # tiles (two SBUF inputs of one DVE op must share a base).
                m1l = mt.tile([64, 512], bf16, tag="m1l", name="m1l")
                m1h = mt.tile([64, 512], bf16, tag="m1h", name="m1h")
                m2l = mt.tile([64, 512], bf16, tag="m2l", name="m2l")
                m2h = mt.tile([64, 512], bf16, tag="m2h", name="m2h")
                nc.vector.tensor_mul(m1l[:], src_bf[0:64, :], t1_sb[0:64, cols])
                nc.vector.tensor_mul(m1h[:], src_bf[64:128, :], t1_sb[64:128, cols])
                nc.vector.tensor_mul(m2l[:], src_bf[0:64, :], t2_sb[0:64, cols])
                nc.vector.tensor_mul(m2h[:], src_bf[64:128, :], t2_sb[64:128, cols])
                nc.vector.tensor_sub(out_even, m1l[:], m1h[:])
                nc.vector.tensor_add(out_odd, m2l[:], m2h[:])

            def emit_fin(fin, psS):
                """PE part of the finalize for a finished chunk: broadcast
                the reciprocal rows, normalize, store, fire the AllGather.
                Deferred into the next chunk's idx==1 slot (or the batch
                tail) so the DVE reciprocal chain never stalls the PE."""
                b_f, t_f, pav_f, rcb_f = fin
                oav = ov.tile([P, 2, 512], bf16, tag="oav", name="oav")
                for h in range(2):
                    rcp_ps = psS.tile([P, 512], f32, tag="ps", name="rb")
                    nc.tensor.matmul(rcp_ps[:], ones_row[:], rcb_f[h][:],
                                     start=True, stop=True)
                    rcp_b = mt.tile([P, 512], bf16, tag="rcpb", name="rcpb")
                    nc.scalar.copy(rcp_b[:], rcp_ps[:])
                    nc.vector.tensor_mul(oav[:, h, :], pav_f[h][:], rcp_b[:])
                for h in range(2):
                    nc.sync.dma_start(
                        ag_in[b_f][t_f][h * P:(h + 1) * P, :], oav[:, h, :])
                nc.gpsimd.collective_compute(
                    "AllGather",
                    mybir.AluOpType.bypass,
                    replica_groups=[list(range(NC))],
                    ins=[ag_in[b_f][t_f].opt()],
                    outs=[ag_out[b_f][t_f].opt()],
                )
                fired.append((b_f, t_f))
                wo_pending.append((b_f, t_f))

            def make_eacc():
                # exp-sum accumulators on DVE in bf16 2x mode (frees ~160 PE
                # matmuls; the PE runs under a ~50% duty-cap governor on this
                # fleet, so total PE work is the wall-clock lever)
                ea = [mt.tile([P, 512], bf16, tag=f"ea{h}", name=f"ea{h}")
                      for h in range(2)]
                for h in range(2):
                    nc.vector.memset(ea[h][:], 0.0)
                return ea

            def attn_chunk(b, t, psS, psV, psD, pending, prealloc, last,
                           wo_min=None):
                il = slice(t * 512, (t + 1) * 512)
                pav = [psV.tile([P, 512], f32, tag=f"pav{h}", name=f"pav{h}")
                       for h in range(2)]
                eacc = prealloc if prealloc is not None else make_eacc()
                nxt = None
                nj = 4 * t + 4
                order = list(range(nj - 1, -1, -1))
                pipe = []

                def pop_av(ep, ip, jp, qs):
                    for h in range(2):
                        nc.tensor.matmul(
                            pav[h][:, qs:], vnat[:, jp, :], ep[h][:, qs:],
                            start=(ip == 0), stop=(ip == nj - 1),
                            skip_group_check=True,
                        )

                for idx, j in enumerate(order):
                    rel = j - 4 * t
                    # columns < 128*rel of a diagonal block are fully masked:
                    # skip them in scores/exp/sum/AV (their pav contribution
                    # comes from other blocks; per-element has_written
                    # semantics keep the accumulation correct)
                    qs = max(rel, 0) * 128
                    epair = []
                    for h in range(2):
                        ps = psS.tile([P, 512], f32, tag="ps", name="ps")
                        if rel >= 0:
                            nc.tensor.matmul(
                                ps[:, qs:qs + 128], ident[:],
                                mskb_sb[:, rel * 512 + qs: rel * 512 + qs + 128],
                                start=True, stop=False,
                            )
                        nc.tensor.matmul(
                            ps[:, qs:], krot[:, j * P:(j + 1) * P],
                            qrot[:, h, slice(t * 512 + qs, (t + 1) * 512)],
                            start=(rel < 0), stop=True,
                            skip_group_check=True,
                        )
                        e = et.tile([P, 512], bf16, tag="e", name="e")
                        nc.scalar.activation(e[:, qs:], ps[:, qs:], Exp)
                        nc.vector.tensor_add(eacc[h][:, qs:], eacc[h][:, qs:],
                                             e[:, qs:])
                        epair.append(e)
                    pipe.append((epair, idx, j, qs))
                    if idx == 1 and pending is not None:
                        emit_fin(pending, psS)
                        pending = None
                    if idx == 2 and not last:
                        # next chunk's accumulators zeroed here so the DVE
                        # work sits mid-chunk, not on the boundary chain
                        nxt = make_eacc()
                    if idx == 3 and wo_min is not None:
                        drain_wo(wo_min)
                    if len(pipe) > 3:
                        ep, ip, jp, qsp = pipe.pop(0)
                        pop_av(ep, ip, jp, qsp)
                for ep, ip, jp, qsp in pipe:
                    pop_av(ep, ip, jp, qsp)
                pipe = []
                # denominators: one ones^T @ eacc matmul per head, then the
                # DVE reciprocal chain ([1,512] reciprocal costs ~3.3us);
                # the PE part of the finalize is deferred into the next
                # chunk (or batch tail)
                pden = [psD.tile([P, 512], f32, tag=f"pd{h}", name=f"pd{h}")
                        for h in range(2)]
                rcb = []
                for h in range(2):
                    nc.tensor.matmul(pden[h][0:1, :], ones_sb[:], eacc[h][:],
                                     start=True, stop=True)
                for h in range(2):
                    # approx reciprocal: ~5x faster than reciprocal() at
                    # ~18 correct bits (plenty for the bf16 broadcast);
                    # denominators are strictly positive, no edge cases
                    rcf = rt.tile([1, 512], f32, tag="rcf", name="rcf")
                    nc.vector.reciprocal_approx_fast(
                        out=rcf[:], in_=pden[h][0:1, :])
                    rcb_h = rt.tile([1, 512], bf16, tag=f"rcb{h}", name=f"rcb{h}")
                    nc.vector.tensor_copy(rcb_h[:], rcf[:])
                    rcb.append(rcb_h)
                return (b, t, pav, rcb), nxt

            # ---- phases ----
            # b0 runs its chunks small-first so its AllGather pipeline starts
            # as early as possible (the wo work lands in proj(b1)'s windows);
            # b1 runs big-first so the tail only waits on small-chunk AGs.
            for b, torder, wo_min in ((0, (0, 1, 2, 3), None),
                                      (1, (3, 2, 1, 0), None)):
                with (
                    tc.tile_pool(name=f"psA2{b}", bufs=1, space="PSUM") as psA2,
                    tc.tile_pool(name=f"psA1{b}", bufs=1, space="PSUM") as psA1,
                    tc.tile_pool(name=f"psT{b}", bufs=2, space="PSUM") as psT,
                ):
                    proj_batch(b, psA2, psA1, psT, wo_lag=2 if b else None)
                if b == 0:
                    # wo weights aren't needed until proj(b1); loading here
                    # keeps them out of the prologue DMA burst
                    nc.gpsimd.dma_start(
                        wo_sb[:], wo_h.rearrange("p (mb d) -> p mb d", mb=MB))
                with (
                    tc.tile_pool(name=f"psS{b}", bufs=3, space="PSUM") as psS,
                    tc.tile_pool(name=f"psV{b}", bufs=1, space="PSUM") as psV,
                    tc.tile_pool(name=f"psD{b}", bufs=1, space="PSUM") as psD,
                ):
                    pending = None
                    prealloc = None
                    for ti, t in enumerate(torder):
                        pending, prealloc = attn_chunk(
                            b, t, psS, psV, psD, pending, prealloc,
                            last=(ti == 3), wo_min=wo_min)
                    emit_fin(pending, psS)
            while wo_pending:
                emit_wo_one()

    nc.compile()
    return nc


def _prep_inputs(x, freqs_cos, freqs_sin, wq, wk, wv, wo):
    x = np.asarray(x, np.float32).reshape(NS, DIM)
    xT = np.ascontiguousarray(
        x.T.reshape(MB, P, 8, 512).transpose(0, 2, 1, 3)).astype(BF)
    cos = np.tile(np.asarray(freqs_cos, np.float32), (B, 1)).T  # (64, NS)
    sin = np.tile(np.asarray(freqs_sin, np.float32), (B, 1)).T
    trig1 = np.ascontiguousarray(np.concatenate([cos, sin], axis=0)).astype(BF)
    trig2 = np.ascontiguousarray(np.concatenate([sin, cos], axis=0)).astype(BF)

    perm = np.r_[np.arange(0, HD, 2), np.arange(1, HD, 2)]
    scale = np.float32(1.0 / np.sqrt(HD))
    wq = np.asarray(wq, np.float32) * scale
    wk = np.asarray(wk, np.float32)
    wv = np.asarray(wv, np.float32)
    wo = np.asarray(wo, np.float32)

    masks = np.zeros((P, 4, 512), np.float32)
    for p in range(4):
        for isub in range(4):
            sl = slice(isub * 128, (isub + 1) * 128)
            if p < isub:
                masks[:, p, sl] = 1.0
            elif p == isub:
                masks[:, p, sl] = np.triu(np.ones((P, P), np.float32))
    maskb = np.ascontiguousarray(
        (-30.0 * (1.0 - masks)).reshape(P, 4 * 512)).astype(BF)

    def tile_w(w):
        d = w.shape[1]
        return np.ascontiguousarray(
            w.reshape(MB, P, d).transpose(1, 0, 2).reshape(P, MB * d)).astype(BF)

    in_maps = []
    for c in range(NC):
        wq_c = wq[:, c * 256:(c + 1) * 256]
        wq_cp = np.concatenate([wq_c[:, h * HD + perm] for h in range(2)], axis=1)
        in_maps.append({
            "xT": xT,
            "wq_c": tile_w(wq_cp),
            "wk_c": tile_w(wk[:, c * HD:(c + 1) * HD][:, perm]),
            "wv_c": tile_w(wv[:, c * HD:(c + 1) * HD]),
            "wo_c": tile_w(wo[:, c * 256:(c + 1) * 256]),
            "trig1": trig1,
            "trig2": trig2,
            "maskb": maskb,
        })
    return in_maps


def _run(inputs, trace=False, **kw):
    from concourse.bass_utils import run_bass_kernel_spmd

    if "nc" not in _cache:
        _cache["nc"] = _build()
    nc = _cache["nc"]
    in_maps = _prep_inputs(**inputs)
    res = run_bass_kernel_spmd(
        nc, in_maps, core_ids=list(range(NC)), trace=trace, **kw
    )
    out = np.empty((NS, DIM), np.float32)
    for c in range(NC):
        out[:, c * 256:(c + 1) * 256] = res.results[c]["outT"].astype(np.float32).T
    return out.reshape(B, S, DIM), res


def kernel(**inputs) -> np.ndarray:
    out, _ = _run(inputs, trace=False)
    return out


# revision 57
# speedup vs baseline: 1.0213x; 1.0213x over previous
"""Trainium2 Bass kernel for GQA attention (B=2, S=2048, DIM=2048, H=16, KV=8,
HD=128) with RoPE + causal mask + output projection.

Sharding: 8-way tensor parallelism over heads. Core c computes q heads
{2c, 2c+1} and kv head c end-to-end (QKV projection, RoPE, causal attention),
contributes its transposed attention output to on-device AllGathers, then
computes the output-projection column slice out[:, 256c:256(c+1)] from the
gathered activations. The host only slices inputs and concatenates outputs.

Schedule (v6; driven by trace analysis - NOTE this fleet runs the PE under
a ~50% duty-cap governor, so total PE work, not packing, sets wall-clock):

  proj(b0) -> attn(b0: t0..t3, AG fired per chunk, finalize deferred one
  chunk) -> proj(b1) [wo(b0 chunks) interleaved between windows]
  -> attn(b1: t3..t0) -> tail wo(b1 chunks)

The first AllGather fires right after proj(b0)+one small chunk (~110us) and
the 8 AGs overlap proj(b1)/attn(b1). b0 runs small-first so its AG pipeline
starts earliest; b1 runs big-first so the tail waits only small-chunk AGs.

Key points:
- softmax denominator = DVE bf16 2x-mode accumulation of exp tiles (eacc),
  then one ones^T @ eacc PE matmul per head into a dedicated PSUM bank row
  (a shared-bank row-offset variant is numerically broken - micro_psum.py).
  The exact [1,512] DVE reciprocal costs ~3.3us (cost scales with elements,
  so a broadcast-then-[128,512]-reciprocal variant is far worse);
  reciprocal_approx_fast (~18 correct bits, ~5x faster) is used instead,
  and the finalize (PE row-broadcast, ACT copy, DVE normalize, AG fire) is
  deferred into the next chunk's idx==1 slot to cover the chain.
- causal-masked columns of diagonal blocks are skipped end-to-end: scores /
  exp / eacc-add / AV all operate on the [128*rel:512] column subset
  (per-element PSUM has_written semantics keep the accumulations correct;
  micro_psum.py test B validates the subset-start pattern).
- causal mask as subset-width -30 bias matmuls before the scores matmul
  (start=True clears the whole bank).
- RoPE as 6 bf16 2x-mode DVE ops per head on ACT-evicted bf16 copies of
  the projection PSUM, with [cos;sin]/[sin;cos] stacked trig tables.
- quarter-batched DMAs (one trigger per [128,4,512]): xt stream on the sync
  queue, gathered wo chunks on the scalar queue. Engine queues are kept
  decoupled: POOL carries only AG doorbells, ag_in stores ride sync, out
  stores ride scalar - anything after a doorbell on POOL waits on the
  collective and must not gate compute.
- AG outputs addr_space="Shared"; output stored bf16, host upcasts.
- single set of qrot/krot/vnat buffers reused by both batches (b0's
  attention completes before proj(b1) overwrites them).
- eacc accumulators for chunk i+1 are allocated+zeroed mid-chunk-i on DVE
  so the memsets sit off the chunk-boundary dependency chain.
"""

import sys

if "/opt/trn_rl_repo" not in sys.path:
    sys.path.insert(0, "/opt/trn_rl_repo")

import numpy as np
import ml_dtypes

B, S, DIM = 2, 2048, 2048
H, KV, HD = 16, 8, 128
NC = 8
NS = B * S            # 4096 flattened (b, s) rows
P = 128
MB = DIM // P         # 16 contraction blocks for the projections
BF = ml_dtypes.bfloat16

_cache: dict = {}


def _build(debug=False):
    import concourse.bass as bass
    import concourse.mybir as mybir
    import concourse.tile as tile
    from concourse import bacc
    from concourse.masks import make_identity

    dt = mybir.dt
    f32, bf16 = dt.float32, dt.bfloat16
    Exp = mybir.ActivationFunctionType.Exp

    nc = bacc.Bacc("TRN2", debug=False, target_bir_lowering=False, num_devices=NC)

    xT_h = nc.dram_tensor("xT", (MB, 8, P, 512), bf16, kind="ExternalInput").ap()
    wq_h = nc.dram_tensor("wq_c", (P, MB * 256), bf16, kind="ExternalInput").ap()
    wk_h = nc.dram_tensor("wk_c", (P, MB * HD), bf16, kind="ExternalInput").ap()
    wv_h = nc.dram_tensor("wv_c", (P, MB * HD), bf16, kind="ExternalInput").ap()
    wo_h = nc.dram_tensor("wo_c", (P, MB * 256), bf16, kind="ExternalInput").ap()
    # stacked trig tables: T1 = [cos; sin], T2 = [sin; cos]  (128 x NS)
    t1_h = nc.dram_tensor("trig1", (P, NS), bf16, kind="ExternalInput").ap()
    t2_h = nc.dram_tensor("trig2", (P, NS), bf16, kind="ExternalInput").ap()
    mskb_h = nc.dram_tensor("maskb", (P, 4 * 512), bf16, kind="ExternalInput").ap()
    out_h = nc.dram_tensor("outT", (256, NS), bf16, kind="ExternalOutput").ap()

    with tile.TileContext(nc) as tc:
        with (
            tc.tile_pool(name="const", bufs=1) as const,
            tc.tile_pool(name="persist", bufs=1) as persist,
            tc.tile_pool(name="xs", bufs=2) as xs,
            tc.tile_pool(name="vt", bufs=2) as vt,
            tc.tile_pool(name="mt", bufs=2) as mt,
            tc.tile_pool(name="et", bufs=20) as et,
            tc.tile_pool(name="rt", bufs=2) as rt,
            tc.tile_pool(name="ov", bufs=2) as ov,
            tc.tile_pool(name="gp", bufs=2) as gp,
            tc.tile_pool(name="ot", bufs=3) as ot,
            tc.tile_pool(name="dram", bufs=1, space="DRAM") as dram,
            tc.tile_pool(name="psW", bufs=1, space="PSUM") as psW,
        ):
            # ---- constants into SBUF ----
            # wq in halves: the second half loads behind the first xt
            # quarters so window 0's matmuls start ~8us earlier
            wq_sb = [const.tile([P, 8, 256], bf16, name=f"wq{i}")
                     for i in range(2)]
            nc.sync.dma_start(
                wq_sb[0][:],
                wq_h[:, 0:8 * 256].rearrange("p (mb d) -> p mb d", mb=8))
            wk_sb = const.tile([P, MB, HD], bf16)
            nc.scalar.dma_start(wk_sb[:], wk_h.rearrange("p (mb d) -> p mb d", mb=MB))
            wv_sb = const.tile([P, MB, HD], bf16)
            nc.gpsimd.dma_start(wv_sb[:], wv_h.rearrange("p (mb d) -> p mb d", mb=MB))
            t1_sb = const.tile([P, NS], bf16)
            nc.scalar.dma_start(t1_sb[:], t1_h)
            t2_sb = const.tile([P, NS], bf16)
            nc.scalar.dma_start(t2_sb[:], t2_h)
            wo_sb = const.tile([P, MB, 256], bf16)
            mskb_sb = const.tile([P, 4 * 512], bf16)
            nc.gpsimd.dma_start(mskb_sb[:], mskb_h)
            ones_sb = const.tile([P, 1], bf16)
            nc.gpsimd.memset(ones_sb[:], 1.0)
            ones_row = const.tile([1, P], bf16)
            nc.gpsimd.memset(ones_row[:], 1.0)
            ident = const.tile([P, P], bf16)
            make_identity(nc, ident[:])

            # ---- single activation set, reused by both batches ----
            qrot = persist.tile([P, 2, S], bf16, name="qrot")
            krot = persist.tile([P, S], bf16, name="krot")
            vnat = persist.tile([P, S // P, HD], bf16, name="vnat")
            ag_in = [[dram.tile([256, 512], bf16, name=f"agi{b}{t}")
                      for t in range(4)] for b in range(B)]
            ag_out = [[dram.tile([NC * 256, 512], bf16, name=f"ago{b}{t}",
                                 addr_space="Shared")
                       for t in range(4)] for b in range(B)]

            fired = []       # chunks whose AllGather has been emitted
            wo_pending = []  # chunks gathered but whose wo is not yet emitted

            def emit_wo_one():
                """Output-projection slice for the oldest gathered chunk."""
                b_w, t_w = wo_pending.pop(0)
                # quarter loads so the first wo matmuls start after 512KB
                gq = []
                src = ag_out[b_w][t_w].rearrange("(r p) q -> p r q", p=P)
                for q4 in range(4):
                    g = gp.tile([P, 4, 512], bf16, tag=f"g{q4}", name=f"g{q4}")
                    nc.scalar.dma_start(g[:], src[:, q4 * 4:(q4 + 1) * 4])
                    gq.append(g)
                # two passes over one PSUM bank (PSUM budget is 8 banks)
                for n in range(2):
                    pw = psW.tile([P, 512], f32, tag="pw", name="pw")
                    for r in range(MB):
                        nc.tensor.matmul(
                            pw[:], wo_sb[:, r, n * 128:(n + 1) * 128],
                            gq[r // 4][:, r % 4, :],
                            start=(r == 0), stop=(r == MB - 1),
                        )
                    o = ot.tile([P, 512], bf16, tag="o", name="o")
                    nc.vector.tensor_copy(o[:], pw[:])
                    nc.scalar.dma_start(
                        out_h[n * P:(n + 1) * P,
                              b_w * S + t_w * 512: b_w * S + (t_w + 1) * 512],
                        o[:],
                    )

            def drain_wo(min_pending):
                """Emit at most one wo block, only if more than min_pending
                chunks are queued (i.e. the head chunk's AllGather has had
                min_pending chunks of compute time to complete)."""
                if len(wo_pending) > min_pending:
                    emit_wo_one()

            def proj_batch(b, psA2, psA1, psT, wo_lag=None):
                for sp in range(4):          # 512-col windows within batch
                    gw = slice(b * S + sp * 512, b * S + (sp + 1) * 512)
                    lw = slice(sp * 512, (sp + 1) * 512)
                    # quarter loads so matmuls stream behind the DMA
                    xsrc = xT_h[:, b * 4 + sp].rearrange("m p q -> p m q")
                    xtq = []
                    for q4 in range(4):
                        xt = xs.tile([P, 4, 512], bf16, tag=f"xt{q4}",
                                     name=f"xt{q4}")
                        nc.sync.dma_start(xt[:], xsrc[:, q4 * 4:(q4 + 1) * 4])
                        xtq.append(xt)
                        if b == 0 and sp == 0 and q4 == 1:
                            nc.sync.dma_start(
                                wq_sb[1][:],
                                wq_h[:, 8 * 256:].rearrange(
                                    "p (mb d) -> p mb d", mb=8))
                    pq = [psA2.tile([P, 512], f32, tag=f"pq{h}", name=f"pq{h}")
                          for h in range(2)]
                    pk = psA1.tile([P, 512], f32, tag="pk", name="pk")
                    pv = psA1.tile([P, 512], f32, tag="pv", name="pv")
                    for m in range(MB):
                        wq_m = wq_sb[m // 8]
                        for acc, lhsT in (
                            (pq[0], wq_m[:, m % 8, 0:128]),
                            (pq[1], wq_m[:, m % 8, 128:256]),
                            (pk, wk_sb[:, m, :]),
                            (pv, wv_sb[:, m, :]),
                        ):
                            nc.tensor.matmul(
                                acc[:], lhsT, xtq[m // 4][:, m % 4, :],
                                start=(m == 0), stop=(m == MB - 1),
                            )
                    # PSUM -> SBUF bf16 evictions on ACT
                    pq_bf = [mt.tile([P, 512], bf16, tag=f"pqb{h}",
                                     name=f"pqb{h}") for h in range(2)]
                    pk_bf = mt.tile([P, 512], bf16, tag="pkb", name="pkb")
                    for h in range(2):
                        nc.scalar.copy(pq_bf[h][:], pq[h][:])
                    nc.scalar.copy(pk_bf[:], pk[:])
                    vT_w = vt.tile([P, 512], bf16, tag="vtw", name="vtw")
                    nc.scalar.copy(vT_w[:], pv[:])
                    # RoPE on DVE (all-bf16 2x ops)
                    for h in range(2):
                        rope_unit(pq_bf[h], gw,
                                  qrot[0:64, h, lw], qrot[64:128, h, lw])
                    rope_unit(pk_bf, gw, krot[0:64, lw], krot[64:128, lw])
                    # v natural layout: PE transposes of this window's blocks
                    for i4 in range(4):
                        blk = sp * 4 + i4
                        pt = psT.tile([P, P], bf16, tag="pt", name="pt")
                        nc.tensor.transpose(
                            pt[:], vT_w[:, i4 * P:(i4 + 1) * P], ident[:])
                        nc.vector.tensor_copy(vnat[:, blk, :], pt[:])
                    if wo_lag is not None:
                        drain_wo(0)

            def rope_unit(src_bf, cols, out_even, out_odd):
                # src_bf rows 0:64 = even dims E, 64:128 = odd dims O.
                # rotE = E*cos - O*sin, rotO = E*sin + O*cos.
                # t1 = [cos;sin], t2 = [sin;cos]; products go to base-0
                # tiles (two SBUF inputs of one DVE op must share a base).
                m1l = mt.tile([64, 512], bf16, tag="m1l", name="m1l")
                m1h = mt.tile([64, 512], bf16, tag="m1h", name="m1h")
                m2l = mt.tile([64, 512], bf16, tag="m2l", name="m2l")
                m2h = mt.tile([64, 512], bf16, tag="m2h", name="m2h")
                nc.vector.tensor_mul(m1l[:], src_bf[0:64, :], t1_sb[0:64, cols])
                nc.vector.tensor_mul(m1h[:], src_bf[64:128, :], t1_sb[64:128, cols])
                nc.vector.tensor_mul(m2l[:], src_bf[0:64, :], t2_sb[0:64, cols])
                nc.vector.tensor_mul(m2h[:], src_bf[64:128, :], t2_sb[64:128, cols])
                nc.vector.tensor_sub(out_even, m1l[:], m1h[:])
                nc.vector.tensor_add(out_odd, m2l[:], m2h[:])

            def emit_fin(fin, psS):
                """PE part of the finalize for a finished chunk: broadcast
                the reciprocal rows, normalize, store, fire the AllGather.
                Deferred into the next chunk's idx==1 slot (or the batch
                tail) so the DVE reciprocal chain never stalls the PE."""
                b_f, t_f, pav_f, rcb_f = fin
                oav = ov.tile([P, 2, 512], bf16, tag="oav", name="oav")
                for h in range(2):
                    rcp_ps = psS.tile([P, 512], f32, tag="ps", name="rb")
                    nc.tensor.matmul(rcp_ps[:], ones_row[:], rcb_f[h][:],
                                     start=True, stop=True)
                    rcp_b = mt.tile([P, 512], bf16, tag="rcpb", name="rcpb")
                    nc.scalar.copy(rcp_b[:], rcp_ps[:])
                    nc.vector.tensor_mul(oav[:, h, :], pav_f[h][:], rcp_b[:])
                for h in range(2):
                    nc.sync.dma_start(
                        ag_in[b_f][t_f][h * P:(h + 1) * P, :], oav[:, h, :])
                nc.gpsimd.collective_compute(
                    "AllGather",
                    mybir.AluOpType.bypass,
                    replica_groups=[list(range(NC))],
                    ins=[ag_in[b_f][t_f].opt()],
                    outs=[ag_out[b_f][t_f].opt()],
                )
                fired.append((b_f, t_f))
                wo_pending.append((b_f, t_f))

            def make_eacc():
                # exp-sum accumulators on DVE in bf16 2x mode (frees ~160 PE
                # matmuls; the PE runs under a ~50% duty-cap governor on this
                # fleet, so total PE work is the wall-clock lever)
                ea = [mt.tile([P, 512], bf16, tag=f"ea{h}", name=f"ea{h}")
                      for h in range(2)]
                for h in range(2):
                    nc.vector.memset(ea[h][:], 0.0)
                return ea

            def attn_chunk(b, t, psS, psV, psD, pending, prealloc, last,
                           wo_min=None):
                il = slice(t * 512, (t + 1) * 512)
                pav = [psV.tile([P, 512], f32, tag=f"pav{h}", name=f"pav{h}")
                       for h in range(2)]
                eacc = prealloc if prealloc is not None else make_eacc()
                nxt = None
                nj = 4 * t + 4
                order = list(range(nj - 1, -1, -1))
                pipe = []

                def pop_av(ep, ip, jp, qs):
                    for h in range(2):
                        nc.tensor.matmul(
                            pav[h][:, qs:], vnat[:, jp, :], ep[h][:, qs:],
                            start=(ip == 0), stop=(ip == nj - 1),
                            skip_group_check=True,
                        )

                for idx, j in enumerate(order):
                    rel = j - 4 * t
                    # columns < 128*rel of a diagonal block are fully masked:
                    # skip them in scores/exp/sum/AV (their pav contribution
                    # comes from other blocks; per-element has_written
                    # semantics keep the accumulation correct)
                    qs = max(rel, 0) * 128
                    epair = []
                    for h in range(2):
                        ps = psS.tile([P, 512], f32, tag="ps", name="ps")
                        if rel >= 0:
                            nc.tensor.matmul(
                                ps[:, qs:qs + 128], ident[:],
                                mskb_sb[:, rel * 512 + qs: rel * 512 + qs + 128],
                                start=True, stop=False,
                            )
                        nc.tensor.matmul(
                            ps[:, qs:], krot[:, j * P:(j + 1) * P],
                            qrot[:, h, slice(t * 512 + qs, (t + 1) * 512)],
                            start=(rel < 0), stop=True,
                            skip_group_check=True,
                        )
                        e = et.tile([P, 512], bf16, tag="e", name="e")
                        nc.scalar.activation(e[:, qs:], ps[:, qs:], Exp)
                        nc.vector.tensor_add(eacc[h][:, qs:], eacc[h][:, qs:],
                                             e[:, qs:])
                        epair.append(e)
                    pipe.append((epair, idx, j, qs))
                    if idx == 1 and pending is not None:
                        emit_fin(pending, psS)
                        pending = None
                    if idx == 2 and not last:
                        # next chunk's accumulators zeroed here so the DVE
                        # work sits mid-chunk, not on the boundary chain
                        nxt = make_eacc()
                    if idx == 3 and wo_min is not None:
                        drain_wo(wo_min)
                    if len(pipe) > 3:
                        ep, ip, jp, qsp = pipe.pop(0)
                        pop_av(ep, ip, jp, qsp)
                for ep, ip, jp, qsp in pipe:
                    pop_av(ep, ip, jp, qsp)
                pipe = []
                # denominators: one ones^T @ eacc matmul per head, then the
                # DVE reciprocal chain ([1,512] reciprocal costs ~3.3us);
                # the PE part of the finalize is deferred into the next
                # chunk (or batch tail)
                pden = [psD.tile([P, 512], f32, tag=f"pd{h}", name=f"pd{h}")
                        for h in range(2)]
                rcb = []
                for h in range(2):
                    nc.tensor.matmul(pden[h][0:1, :], ones_sb[:], eacc[h][:],
                                     start=True, stop=True)
                for h in range(2):
                    # approx reciprocal: ~5x faster than reciprocal() at
                    # ~18 correct bits (plenty for the bf16 broadcast);
                    # denominators are strictly positive, no edge cases
                    rcf = rt.tile([1, 512], f32, tag="rcf", name="rcf")
                    nc.vector.reciprocal_approx_fast(
                        out=rcf[:], in_=pden[h][0:1, :])
                    rcb_h = rt.tile([1, 512], bf16, tag=f"rcb{h}", name=f"rcb{h}")
                    nc.vector.tensor_copy(rcb_h[:], rcf[:])
                    rcb.append(rcb_h)
                return (b, t, pav, rcb), nxt

            # ---- phases ----
            # b0 runs its chunks small-first so its AllGather pipeline starts
            # as early as possible (the wo work lands in proj(b1)'s windows);
            # b1 runs big-first so the tail only waits on small-chunk AGs.
            for b, torder, wo_min in ((0, (0, 1, 2, 3), None),
                                      (1, (3, 2, 1, 0), 2)):
                with (
                    tc.tile_pool(name=f"psA2{b}", bufs=1, space="PSUM") as psA2,
                    tc.tile_pool(name=f"psA1{b}", bufs=1, space="PSUM") as psA1,
                    tc.tile_pool(name=f"psT{b}", bufs=2, space="PSUM") as psT,
                ):
                    proj_batch(b, psA2, psA1, psT, wo_lag=2 if b else None)
                if b == 0:
                    # wo weights aren't needed until proj(b1); loading here
                    # keeps them out of the prologue DMA burst
                    nc.gpsimd.dma_start(
                        wo_sb[:], wo_h.rearrange("p (mb d) -> p mb d", mb=MB))
                with (
                    tc.tile_pool(name=f"psS{b}", bufs=3, space="PSUM") as psS,
                    tc.tile_pool(name=f"psV{b}", bufs=1, space="PSUM") as psV,
                    tc.tile_pool(name=f"psD{b}", bufs=1, space="PSUM") as psD,
                ):
                    pending = None
                    prealloc = None
                    for ti, t in enumerate(torder):
                        pending, prealloc = attn_chunk(
                            b, t, psS, psV, psD, pending, prealloc,
                            last=(ti == 3), wo_min=wo_min)
                    emit_fin(pending, psS)
            while wo_pending:
                emit_wo_one()

    nc.compile()
    return nc


def _prep_inputs(x, freqs_cos, freqs_sin, wq, wk, wv, wo):
    x = np.asarray(x, np.float32).reshape(NS, DIM)
    xT = np.ascontiguousarray(
        x.T.reshape(MB, P, 8, 512).transpose(0, 2, 1, 3)).astype(BF)
    cos = np.tile(np.asarray(freqs_cos, np.float32), (B, 1)).T  # (64, NS)
    sin = np.tile(np.asarray(freqs_sin, np.float32), (B, 1)).T
    trig1 = np.ascontiguousarray(np.concatenate([cos, sin], axis=0)).astype(BF)
    trig2 = np.ascontiguousarray(np.concatenate([sin, cos], axis=0)).astype(BF)

    perm = np.r_[np.arange(0, HD, 2), np.arange(1, HD, 2)]
    scale = np.float32(1.0 / np.sqrt(HD))
    wq = np.asarray(wq, np.float32) * scale
    wk = np.asarray(wk, np.float32)
    wv = np.asarray(wv, np.float32)
    wo = np.asarray(wo, np.float32)

    masks = np.zeros((P, 4, 512), np.float32)
    for p in range(4):
        for isub in range(4):
            sl = slice(isub * 128, (isub + 1) * 128)
            if p < isub:
                masks[:, p, sl] = 1.0
            elif p == isub:
                masks[:, p, sl] = np.triu(np.ones((P, P), np.float32))
    maskb = np.ascontiguousarray(
        (-30.0 * (1.0 - masks)).reshape(P, 4 * 512)).astype(BF)

    def tile_w(w):
        d = w.shape[1]
        return np.ascontiguousarray(
            w.reshape(MB, P, d).transpose(1, 0, 2).reshape(P, MB * d)).astype(BF)

    in_maps = []
    for c in range(NC):
        wq_c = wq[:, c * 256:(c + 1) * 256]
        wq_cp = np.concatenate([wq_c[:, h * HD + perm] for h in range(2)], axis=1)
        in_maps.append({
            "xT": xT,
            "wq_c": tile_w(wq_cp),
            "wk_c": tile_w(wk[:, c * HD:(c + 1) * HD][:, perm]),
            "wv_c": tile_w(wv[:, c * HD:(c + 1) * HD]),
            "wo_c": tile_w(wo[:, c * 256:(c + 1) * 256]),
            "trig1": trig1,
            "trig2": trig2,
            "maskb": maskb,
        })
    return in_maps


def _run(inputs, trace=False, **kw):
    from concourse.bass_utils import run_bass_kernel_spmd

    if "nc" not in _cache:
        _cache["nc"] = _build()
    nc = _cache["nc"]
    in_maps = _prep_inputs(**inputs)
    res = run_bass_kernel_spmd(
        nc, in_maps, core_ids=list(range(NC)), trace=trace, **kw
    )
    out = np.empty((NS, DIM), np.float32)
    for c in range(NC):
        out[:, c * 256:(c + 1) * 256] = res.results[c]["outT"].astype(np.float32).T
    return out.reshape(B, S, DIM), res


def kernel(**inputs) -> np.ndarray:
    out, _ = _run(inputs, trace=False)
    return out
